# revision 26
# baseline (speedup 1.0000x reference)
"""Trainium2 Bass kernel for blocked-DCT high-frequency extractor.

v7 structure (best measured pacing: per-super DMAs, per-tile compute,
output DMAs on the SP queue which self-paces the input stream) plus:
fp8 e3m4 output (x8 scale in the stationary), tensor_copy u8->bf16
widen, bf16 matmul moving operand.

See kernel.py history for the full design rationale:
* q_c = rint(x_c * w_c * 255) per-channel quantization -> gray is a
  plain byte sum (max 255): packed u16-lane adds on DVE, no carries.
* One 64->48 masked-DCT stationary per 8x8 block, [128, 96] block-diag
  processing two 1024-block halves per tile; masked coefficients never
  computed, host scatters zeros while widening fp8 -> f32.
"""

import os

import ml_dtypes
import numpy as np

import concourse.bacc as bacc
import concourse.mybir as mybir
import concourse.tile as tile
from concourse.bass_utils import run_bass_kernel_spmd

N_CORES = 8
B, C, H, W = 64, 3, 512, 512
BLOC = B // N_CORES          # images per core
NT = 16                      # tiles per core
NS = 8                       # super-tiles per core (2 tiles each)
BLK = 2048                   # 8x8 blocks per tile
P = 128
BF16 = mybir.dt.bfloat16
F32 = mybir.dt.float32
U8 = mybir.dt.uint8
U16 = mybir.dt.uint16
F8E3 = mybir.dt.float8e3
GRAY_W = (0.299, 0.587, 0.114)
KEPT = [il for il in range(64) if not (il // 8 < 4 and il % 8 < 4)]
ALU = mybir.AluOpType
OUT_SCALE = 8.0

_NC = None
LAST_RUN = None


def _build_bass():
    nc = bacc.Bacc(
        "TRN2",
        target_bir_lowering=False,
        debug=False,
        num_devices=N_CORES,
    )
    xrg = nc.declare_dram_parameter("xrg", [NS, P, 4096], U8, isOutput=False)
    xb = nc.declare_dram_parameter("xb", [NS, P, 2048], U8, isOutput=False)
    wts = nc.declare_dram_parameter("wts", [P, 96], BF16, isOutput=False)
    out = nc.declare_dram_parameter("out", [NS, 96, 2048], F8E3,
                                    isOutput=True)

    with tile.TileContext(nc) as tc:
        with (
            tc.tile_pool(name="consts", bufs=1) as consts,
            tc.tile_pool(name="xin", bufs=3) as xin_pool,
            tc.tile_pool(name="bin", bufs=3) as bin_pool,
            tc.tile_pool(name="s1p", bufs=2) as s1_pool,
            tc.tile_pool(name="s2p", bufs=3) as s2_pool,
            tc.tile_pool(name="widep", bufs=4) as wide_pool,
            tc.tile_pool(name="sout", bufs=3) as sout_pool,
            tc.tile_pool(name="psum", bufs=4, space="PSUM") as psum_pool,
        ):
            wt = consts.tile([P, 96], BF16, tag="wt")
            nc.scalar.dma_start(wt[:], wts[:])
            # tiny priming read absorbs the SP ring spin-up cost so the
            # first real input DMA lands sooner
            prime = consts.tile([P, 64], U8, tag="prime")
            nc.sync.dma_start(prime[:], xrg[0][:, 0:64])

            # work units (super, first tile t0, tile count k): the first
            # and last supers are split into single tiles so the pipeline
            # ramp and drain run on half-size units
            units = ([(0, 0, 1), (0, 1, 1)] +
                     [(u, 0, 2) for u in range(1, NS - 1)] +
                     [(NS - 1, 0, 1), (NS - 1, 1, 1)])
            NU = len(units)
            xts = [None] * NU
            bts = [None] * NU
            wds = [None] * (2 * NU)

            for i in range(NU + 2):
                iD, iV, iM = i, i - 1, i - 2
                # --- SP: R|G input (per-tile (t2, c2, f) cols)
                if iD < NU:
                    u, t0, k = units[iD]
                    xt = xin_pool.tile([P, 4096], U8, tag="xin")
                    nc.sync.dma_start(
                        xt[:, 0:k * 2048],
                        xrg[u][:, t0 * 2048:(t0 + k) * 2048])
                    xts[iD] = xt
                    # --- GpSimd: B input (SWDGE)
                    bt = bin_pool.tile([P, 2048], U8, tag="bin")
                    nc.gpsimd.dma_start(
                        bt[:, 0:k * 1024],
                        xb[u][:, t0 * 1024:(t0 + k) * 1024])
                    bts[iD] = bt
                # --- DVE per tile: packed u16 adds + tensor_copy widen
                if 0 <= iV < NU:
                    _, _, k = units[iV]
                    for t2 in range(k):
                        o = t2 * 2048
                        s1 = s1_pool.tile([P, 1024], U8, tag="s1")
                        nc.vector.tensor_tensor(
                            s1[:].bitcast(U16),
                            xts[iV][:, o:o + 1024].bitcast(U16),
                            xts[iV][:, o + 1024:o + 2048].bitcast(U16),
                            ALU.add)
                        s2 = s2_pool.tile([P, 1024], U8, tag="s2")
                        nc.vector.tensor_tensor(
                            s2[:].bitcast(U16), s1[:].bitcast(U16),
                            bts[iV][:, t2 * 1024:(t2 + 1) * 1024].bitcast(U16),
                            ALU.add)
                        wd = wide_pool.tile([P, 1024], BF16, tag="wide")
                        nc.vector.tensor_copy(wd[:], s2[:])
                        wds[2 * iV + t2] = wd
                    xts[iV] = None
                    bts[iV] = None
                # --- TensorE + ACT + SP out (out on SP self-paces input)
                if 0 <= iM < NU:
                    u, t0, k = units[iM]
                    so = sout_pool.tile([96, 2048], F8E3, tag="sout")
                    for t2 in range(k):
                        wd = wds[2 * iM + t2]
                        ps = psum_pool.tile([96, 1024], F32, tag="ps")
                        for bank in range(2):
                            cs = slice(bank * 512, (bank + 1) * 512)
                            nc.tensor.matmul(ps[:, cs], wt[:], wd[:, cs],
                                             start=True, stop=True)
                        wds[2 * iM + t2] = None
                        nc.scalar.copy(
                            so[:, t2 * 1024:(t2 + 1) * 1024], ps[:])
                    nc.sync.dma_start(
                        out[u][:, t0 * 1024:(t0 + k) * 1024],
                        so[:, 0:k * 1024])
    nc.compile()
    return nc


def _host_constants(dct_matrix, mask):
    D = np.asarray(dct_matrix, dtype=np.float64)
    mask = np.asarray(mask, dtype=np.float64)
    K = (mask[:, :, None, None] * np.einsum('ij,lk->iljk', D, D)).reshape(64, 64)
    s48 = K.T[:, KEPT] * (OUT_SCALE / 255.0)
    w = np.zeros((128, 96))
    w[:64, :48] = s48
    w[64:, 48:] = s48
    return w.astype(ml_dtypes.bfloat16)


def _quantize(x):
    s = np.array(GRAY_W, dtype=np.float32).reshape(1, 3, 1, 1) * 255.0
    return np.clip(np.rint(x * s), 0, 255).astype(np.uint8)


def _relayout_input(xq):
    """uint8 -> per-core ([NS,128,4096] R|G cols (t2,c2,f), [NS,128,2048] B)."""
    rgs, bs = [], []
    for cid in range(N_CORES):
        xc = xq[cid * BLOC:(cid + 1) * BLOC]
        a = xc.reshape(BLOC, 3, 64, 8, 64, 8)               # b c r j m k
        a = a.transpose(1, 0, 2, 4, 3, 5).reshape(3, NT * BLK, 64)  # c n jk
        a = a.reshape(3, NT, 2, 1024, 64)                   # c t s f jk
        a = a.transpose(0, 1, 2, 4, 3).reshape(3, NS, 2, 128, 1024)  # c u t2 p f
        rg = a[0:2].transpose(1, 3, 2, 0, 4)                # u p t2 c2 f
        rgs.append(np.ascontiguousarray(rg.reshape(NS, 128, 4096)))
        bb = a[2].transpose(0, 2, 1, 3)                     # u p t2 f
        bs.append(np.ascontiguousarray(bb.reshape(NS, 128, 2048)))
    return rgs, bs


def _unpermute_output(o_dev):
    """[N_CORES, NS, 96, 2048] fp8e3 -> (64, 1, 512, 512) f32."""
    o = np.asarray(o_dev).astype(np.float32) * (1.0 / OUT_SCALE)
    o = o.reshape(N_CORES, NS, 2, 48, 2, 1024)              # c u s a t2 f
    o = o.transpose(0, 1, 4, 2, 5, 3)                       # c u t2 s f a
    z = np.zeros((N_CORES, NT, 2, 1024, 64), dtype=np.float32)
    z[..., KEPT] = o.reshape(N_CORES, NT, 2, 1024, 48)
    z = z.reshape(B, 64, 64, 8, 8)                          # b r m i l
    z = z.transpose(0, 1, 3, 2, 4).reshape(B, 1, H, W)      # b (r i) (m l)
    return np.ascontiguousarray(z)


def kernel(x, dct_matrix, mask):
    global _NC, LAST_RUN
    x = np.asarray(x)
    assert x.shape == (B, C, H, W)
    xq = _quantize(np.asarray(x, dtype=np.float32))
    wts = _host_constants(dct_matrix, mask)

    if _NC is None:
        _NC = _build_bass()

    rgs, bs = _relayout_input(xq)
    in_maps = [{"xrg": rgs[i], "xb": bs[i], "wts": wts}
               for i in range(N_CORES)]
    trace = bool(int(os.environ.get("DCT_TRACE", "0")))
    LAST_RUN = run_bass_kernel_spmd(
        _NC, in_maps, list(range(N_CORES)), trace=trace,
    )
    o_dev = np.stack([LAST_RUN.results[i]["out"] for i in range(N_CORES)])
    return _unpermute_output(o_dev)


# revision 28
# speedup vs baseline: 1.0426x; 1.0426x over previous
"""Trainium2 Bass kernel for blocked-DCT high-frequency extractor.

v7 structure (best measured pacing: per-super DMAs, per-tile compute,
output DMAs on the SP queue which self-paces the input stream) plus:
fp8 e3m4 output (x8 scale in the stationary), tensor_copy u8->bf16
widen, bf16 matmul moving operand.

See kernel.py history for the full design rationale:
* q_c = rint(x_c * w_c * 255) per-channel quantization -> gray is a
  plain byte sum (max 255): packed u16-lane adds on DVE, no carries.
* One 64->48 masked-DCT stationary per 8x8 block, [128, 96] block-diag
  processing two 1024-block halves per tile; masked coefficients never
  computed, host scatters zeros while widening fp8 -> f32.
"""

import os

import ml_dtypes
import numpy as np

import concourse.bacc as bacc
import concourse.mybir as mybir
import concourse.tile as tile
from concourse.bass_utils import run_bass_kernel_spmd

N_CORES = 8
B, C, H, W = 64, 3, 512, 512
BLOC = B // N_CORES          # images per core
NT = 16                      # tiles per core
NS = 8                       # super-tiles per core (2 tiles each)
BLK = 2048                   # 8x8 blocks per tile
P = 128
BF16 = mybir.dt.bfloat16
F32 = mybir.dt.float32
U8 = mybir.dt.uint8
U16 = mybir.dt.uint16
F8E3 = mybir.dt.float8e3
GRAY_W = (0.299, 0.587, 0.114)
KEPT = [il for il in range(64) if not (il // 8 < 4 and il % 8 < 4)]
ALU = mybir.AluOpType
OUT_SCALE = 8.0

_NC = None
LAST_RUN = None


def _build_bass():
    nc = bacc.Bacc(
        "TRN2",
        target_bir_lowering=False,
        debug=False,
        num_devices=N_CORES,
    )
    xrg = nc.declare_dram_parameter("xrg", [NS, P, 4096], U8, isOutput=False)
    xb = nc.declare_dram_parameter("xb", [NS, P, 2048], U8, isOutput=False)
    wts = nc.declare_dram_parameter("wts", [P, 96], BF16, isOutput=False)
    out = nc.declare_dram_parameter("out", [NS, 96, 2048], F8E3,
                                    isOutput=True)

    with tile.TileContext(nc) as tc:
        with (
            tc.tile_pool(name="consts", bufs=1) as consts,
            tc.tile_pool(name="xin", bufs=3) as xin_pool,
            tc.tile_pool(name="bin", bufs=3) as bin_pool,
            tc.tile_pool(name="s1p", bufs=2) as s1_pool,
            tc.tile_pool(name="s2p", bufs=3) as s2_pool,
            tc.tile_pool(name="widep", bufs=4) as wide_pool,
            tc.tile_pool(name="sout", bufs=3) as sout_pool,
            tc.tile_pool(name="psum", bufs=4, space="PSUM") as psum_pool,
        ):
            wt = consts.tile([P, 96], BF16, tag="wt")
            nc.scalar.dma_start(wt[:], wts[:])

            # work units (super, first tile t0, tile count k): the first
            # and last supers are split into single tiles so the pipeline
            # ramp and drain run on half-size units
            units = ([(0, 0, 1), (0, 1, 1), (1, 0, 1), (1, 1, 1)] +
                     [(u, 0, 2) for u in range(2, NS - 1)] +
                     [(NS - 1, 0, 1), (NS - 1, 1, 1)])
            NU = len(units)
            xts = [None] * NU
            bts = [None] * NU
            wds = [None] * (2 * NU)

            for i in range(NU + 2):
                iD, iV, iM = i, i - 1, i - 2
                # --- SP: R|G input (per-tile (t2, c2, f) cols)
                if iD < NU:
                    u, t0, k = units[iD]
                    xt = xin_pool.tile([P, 4096], U8, tag="xin")
                    nc.sync.dma_start(
                        xt[:, 0:k * 2048],
                        xrg[u][:, t0 * 2048:(t0 + k) * 2048])
                    xts[iD] = xt
                    # --- GpSimd: B input (SWDGE)
                    bt = bin_pool.tile([P, 2048], U8, tag="bin")
                    nc.gpsimd.dma_start(
                        bt[:, 0:k * 1024],
                        xb[u][:, t0 * 1024:(t0 + k) * 1024])
                    bts[iD] = bt
                # --- DVE per tile: packed u16 adds + tensor_copy widen
                if 0 <= iV < NU:
                    _, _, k = units[iV]
                    for t2 in range(k):
                        o = t2 * 2048
                        s1 = s1_pool.tile([P, 1024], U8, tag="s1")
                        nc.vector.tensor_tensor(
                            s1[:].bitcast(U16),
                            xts[iV][:, o:o + 1024].bitcast(U16),
                            xts[iV][:, o + 1024:o + 2048].bitcast(U16),
                            ALU.add)
                        s2 = s2_pool.tile([P, 1024], U8, tag="s2")
                        nc.vector.tensor_tensor(
                            s2[:].bitcast(U16), s1[:].bitcast(U16),
                            bts[iV][:, t2 * 1024:(t2 + 1) * 1024].bitcast(U16),
                            ALU.add)
                        wd = wide_pool.tile([P, 1024], BF16, tag="wide")
                        nc.vector.tensor_copy(wd[:], s2[:])
                        wds[2 * iV + t2] = wd
                    xts[iV] = None
                    bts[iV] = None
                # --- TensorE + ACT + SP out (out on SP self-paces input)
                if 0 <= iM < NU:
                    u, t0, k = units[iM]
                    so = sout_pool.tile([96, 2048], F8E3, tag="sout")
                    for t2 in range(k):
                        wd = wds[2 * iM + t2]
                        ps = psum_pool.tile([96, 1024], F32, tag="ps")
                        for bank in range(2):
                            cs = slice(bank * 512, (bank + 1) * 512)
                            nc.tensor.matmul(ps[:, cs], wt[:], wd[:, cs],
                                             start=True, stop=True)
                        wds[2 * iM + t2] = None
                        nc.scalar.copy(
                            so[:, t2 * 1024:(t2 + 1) * 1024], ps[:])
                    nc.sync.dma_start(
                        out[u][:, t0 * 1024:(t0 + k) * 1024],
                        so[:, 0:k * 1024])
    nc.compile()
    return nc


def _host_constants(dct_matrix, mask):
    D = np.asarray(dct_matrix, dtype=np.float64)
    mask = np.asarray(mask, dtype=np.float64)
    K = (mask[:, :, None, None] * np.einsum('ij,lk->iljk', D, D)).reshape(64, 64)
    s48 = K.T[:, KEPT] * (OUT_SCALE / 255.0)
    w = np.zeros((128, 96))
    w[:64, :48] = s48
    w[64:, 48:] = s48
    return w.astype(ml_dtypes.bfloat16)


def _quantize(x):
    s = np.array(GRAY_W, dtype=np.float32).reshape(1, 3, 1, 1) * 255.0
    return np.clip(np.rint(x * s), 0, 255).astype(np.uint8)


def _relayout_input(xq):
    """uint8 -> per-core ([NS,128,4096] R|G cols (t2,c2,f), [NS,128,2048] B)."""
    rgs, bs = [], []
    for cid in range(N_CORES):
        xc = xq[cid * BLOC:(cid + 1) * BLOC]
        a = xc.reshape(BLOC, 3, 64, 8, 64, 8)               # b c r j m k
        a = a.transpose(1, 0, 2, 4, 3, 5).reshape(3, NT * BLK, 64)  # c n jk
        a = a.reshape(3, NT, 2, 1024, 64)                   # c t s f jk
        a = a.transpose(0, 1, 2, 4, 3).reshape(3, NS, 2, 128, 1024)  # c u t2 p f
        rg = a[0:2].transpose(1, 3, 2, 0, 4)                # u p t2 c2 f
        rgs.append(np.ascontiguousarray(rg.reshape(NS, 128, 4096)))
        bb = a[2].transpose(0, 2, 1, 3)                     # u p t2 f
        bs.append(np.ascontiguousarray(bb.reshape(NS, 128, 2048)))
    return rgs, bs


def _unpermute_output(o_dev):
    """[N_CORES, NS, 96, 2048] fp8e3 -> (64, 1, 512, 512) f32."""
    o = np.asarray(o_dev).astype(np.float32) * (1.0 / OUT_SCALE)
    o = o.reshape(N_CORES, NS, 2, 48, 2, 1024)              # c u s a t2 f
    o = o.transpose(0, 1, 4, 2, 5, 3)                       # c u t2 s f a
    z = np.zeros((N_CORES, NT, 2, 1024, 64), dtype=np.float32)
    z[..., KEPT] = o.reshape(N_CORES, NT, 2, 1024, 48)
    z = z.reshape(B, 64, 64, 8, 8)                          # b r m i l
    z = z.transpose(0, 1, 3, 2, 4).reshape(B, 1, H, W)      # b (r i) (m l)
    return np.ascontiguousarray(z)


def kernel(x, dct_matrix, mask):
    global _NC, LAST_RUN
    x = np.asarray(x)
    assert x.shape == (B, C, H, W)
    xq = _quantize(np.asarray(x, dtype=np.float32))
    wts = _host_constants(dct_matrix, mask)

    if _NC is None:
        _NC = _build_bass()

    rgs, bs = _relayout_input(xq)
    in_maps = [{"xrg": rgs[i], "xb": bs[i], "wts": wts}
               for i in range(N_CORES)]
    trace = bool(int(os.environ.get("DCT_TRACE", "0")))
    LAST_RUN = run_bass_kernel_spmd(
        _NC, in_maps, list(range(N_CORES)), trace=trace,
    )
    o_dev = np.stack([LAST_RUN.results[i]["out"] for i in range(N_CORES)])
    return _unpermute_output(o_dev)
